# revision 7
# baseline (speedup 1.0000x reference)
"""Self-attention kernel for Trainium2, SPMD across 8 NeuronCores.

Reference computation (fp32):
    q = x @ Wq + bq; k = x @ Wk + bk; v = x @ Wv + bv
    out = softmax((q @ k.T) / sqrt(d_q), axis=1) @ v

Sharding: rows of Q (sequence dim N=8192) are sharded across the 8
cores (1024 rows each).  K/V are computed redundantly on every core:
in this environment the ncfw collective_compute AllGather measures
~100us+ for 512KB/rank (65us control-plane dead time), and
remote_dma_broadcast moves SBUF->SBUF data but its remote_sem
increments never arrive (waits on them wedge the exec unit), so there
is no usable cross-core completion signal to shard the projections.

Host-side layout: x.T is pre-arranged into 16 token-blocks of shape
[128, 4096], twice: bfloat16 (V/Q projections) and fp8-e4m3 (K
projection).  The block axis is rotated per core so block 0 holds the
core's own Q tokens; the attention j-loop order does not affect the
softmax sums.  Wk is fp8 pre-scaled by 16 (dodges e4m3 denormals; the
1/16 is fused into the K eviction), Wv|Wq are bf16.

Numerics: the K projection runs fp8-e4m3 with MatmulPerfMode.DoubleRow
(two 128-deep k-tiles per pass, 2x PE rate; inapplicable to the
128-deep S/AV matmuls).  fp8 only perturbs the softmax weights -- not
the V values -- so output rel err is ~8.2e-3 (all-bf16: ~2.3e-3; the
2e-2 gate has 2.5x margin; fp8 on V/E is ~3.6% -> fatal).  The softmax
denominator accumulates in fp16 on DVE (2-byte operands hit the DVE 2x
mode: 689 vs 1213ns per [128,1024] add).  PSUM matmuls are 512 wide
(bank-boundary limit).

Per-core dataflow, streamed block by block, attention one block behind:
  - ~20 dummy matmuls during the initial DMA wait pre-warm the PE clock
    and keep it busy until the x-stream flows (execution gaps reset the
    PE p-state ramp; trimming the warmup measurably hurt)
  - K^T[dk, 8192] (fp8 DoubleRow, ACT eviction fusing 1/16 + bias),
    V^T -> V[j, dv] (PE transpose, DVE eviction), Q^T (ACT eviction)
  - per j-tile (128 keys): S^T[kj, qi] = K_tile^T.T @ Q^T (two query
    halves into one 2-bank PSUM tile); one [128,1024] exp on ACT
    (scale=1/sqrt(128); no max subtraction needed -- |scores| < ~3);
    denominator accumulated on DVE in fp16; O^T[dv, qi] += V_tile.T @ E
    accumulated in PSUM over all 64 j-tiles.  V matmuls run one j-tile
    behind the S matmuls so the in-order PE never stalls on exp.
  - epilogue: denominator partition-sum via fp16 ones-matmuls (lands
    per-partition), DVE reciprocal, O^T transposed back 128 rows at a
    time with the 1/den scale fused into the DVE eviction; out-DMAs
    round-robin over the SP and ACT queues.

Engine balance (healthy clock, ~141us): PE ~119us busy -- the
bottleneck (S+AV 55us irreducible at bf16, V-proj ~31us, K-proj 7us);
ACT ~83us; DVE ~70us; ~25MB DMA.  Fixed overheads: ~7us preamble,
~12us tail.  NOTE: the chip clock varies run to run (PE 512-col matmul
216..454ns steady-state observed, 141 vs 170us wall for the same
NEFF); only work-reduction beats that noise.
"""

import numpy as np

import concourse.bacc as bacc
import concourse.mybir as mybir
import concourse.tile as tile
from concourse.bass_utils import run_bass_kernel_spmd
from concourse.masks import make_identity

N_CORES = 8
N = 8192          # sequence length
D = 1024          # d_model
DH = 128          # d_q == d_k == d_v
NB = N // N_CORES # tokens per core (1024)
KT = D // 128     # k-tiles in the contraction over d_model (8)
JBLK = 512        # token block for the K/V projection stream
NJB = N // JBLK   # 16
NJT = N // 128    # 64 j-tiles in the attention loop
QBLK = 512        # query block (fp32 moving-operand max)
NQB = NB // QBLK  # 2
FB = KT * JBLK    # 4096 floats per partition per stream block

F32 = mybir.dt.float32
F16 = mybir.dt.float16
BF16 = mybir.dt.bfloat16
F8 = mybir.dt.float8e4
SCALE = 1.0 / float(np.sqrt(DH))
W8SCALE = 16.0   # Wk is quantized to fp8 pre-scaled by 16 (avoids denormals)

_CACHE = {}

# Results of the last run_bass_kernel_spmd call (for the test harness to
# read exec_time_ns etc. when tracing is enabled via BASS_TRACE).
LAST_RESULTS = None


def _emit(ctx, tc, nc, xT, xT8, wk8, w_all, b_all, out):
    singles = ctx.enter_context(tc.tile_pool(name="singles", bufs=1))
    xt_pool = ctx.enter_context(tc.tile_pool(name="xt", bufs=6))
    x8_pool = ctx.enter_context(tc.tile_pool(name="x8", bufs=6))
    vt_pool = ctx.enter_context(tc.tile_pool(name="vt", bufs=3))
    exp_pool = ctx.enter_context(tc.tile_pool(name="exp", bufs=6))
    oT_pool = ctx.enter_context(tc.tile_pool(name="oT", bufs=3))
    o_pool = ctx.enter_context(tc.tile_pool(name="o", bufs=3))
    ps_pool = ctx.enter_context(tc.tile_pool(name="ps", bufs=2, space="PSUM"))
    pp_pool = ctx.enter_context(tc.tile_pool(name="pp", bufs=2, space="PSUM"))
    po_pool = ctx.enter_context(tc.tile_pool(name="po", bufs=1, space="PSUM"))

    # --- constants / weights ---------------------------------------------
    # w_all layout is (Wv | Wq) bf16; Wk is a separate fp8 tensor (16x
    # pre-scaled, undone in the K eviction) used by DoubleRow matmuls.
    wk8_sb = singles.tile([128, KT, 128], F8, tag="wk8_sb")
    nc.sync.dma_start(out=wk8_sb, in_=wk8)
    w_sb = singles.tile([128, 2 * D], BF16, tag="w_sb")
    nc.sync.dma_start(out=w_sb[:, 0:D], in_=w_all[:, 0:D])
    nc.sync.dma_start(out=w_sb[:, D:2 * D], in_=w_all[:, D:2 * D])
    b_sb = singles.tile([128, 3], F32, tag="b_sb")
    nc.sync.dma_start(out=b_sb, in_=b_all)
    ident_bf = singles.tile([128, 128], BF16, tag="ident_bf")
    ones128 = singles.tile([128, 1], F16, tag="ones128")
    nc.vector.memset(ones128, 1.0)

    W_BASE = {2: 0, 0: D}  # v, q order in w_all

    def w_ap(proj, kt):  # lhsT [128, 128] for projection matmuls
        base = W_BASE[proj] + kt * 128
        return w_sb[:, base:base + 128]

    # --- persistent SBUF tensors -----------------------------------------
    kT_sb = singles.tile([128, N], BF16, tag="kT")    # K^T, all tokens
    v_sb = singles.tile([128, N], BF16, tag="v")      # V natural, 64 j-tiles
    qT_sb = singles.tile([128, NB], BF16, tag="qT")   # Q^T, local tokens
    rden_sb = singles.tile([128, NB // 128], F32, tag="rden")
    acc_all = singles.tile([128, NB], F16, tag="acc_all", name="acc_all")
    po_t = po_pool.tile([128, NB], F32, tag="po", name="po_t")

    def stream_block(jb):
        """DMA block jb and project its K^T / V columns (+ Q^T for jb<2).

        The bf16 x arrives as two half DMAs per block so the V
        projection's first k-tiles start ~1.4us earlier per block.  K is
        projected from the fp8 copy of x with DoubleRow matmuls (two
        128-deep k-tiles per pass, 2x PE rate)."""
        if jb == 0:
            h8a = x8_pool.tile([128, KT // 2, JBLK], F8, tag="x8", name="x80a")
            nc.gpsimd.dma_start(out=h8a, in_=xT8[0, :, 0:FB // 2])
            ha = xt_pool.tile([128, FB // 2], BF16, tag="xt", name="xt0a")
            nc.gpsimd.dma_start(out=ha, in_=xT[0, :, 0:FB // 2])
            h8b = x8_pool.tile([128, KT // 2, JBLK], F8, tag="x8", name="x80b")
            nc.gpsimd.dma_start(out=h8b, in_=xT8[0, :, FB // 2:FB])
            hb = xt_pool.tile([128, FB // 2], BF16, tag="xt", name="xt0b")
            nc.gpsimd.dma_start(out=hb, in_=xT[0, :, FB // 2:FB])
            # identities built here: after block 0's DMA issues (so they
            # don't delay them on gpsimd) but before any transpose reads
            make_identity(nc, ident_bf)
            parts = ((ha, 0), (hb, KT // 2))
            parts8 = ((h8a, 0), (h8b, KT // 2))
        else:
            x8_t = x8_pool.tile([128, KT, JBLK], F8, tag="x8", name=f"x8{jb}")
            nc.gpsimd.dma_start(out=x8_t, in_=xT8[jb])
            # bf16 x in two half DMAs so the V projection's first k-tiles
            # start ~1.4us earlier per block (same pattern as block 0)
            ha = xt_pool.tile([128, FB // 2], BF16, tag="xt", name=f"xt{jb}a")
            nc.gpsimd.dma_start(out=ha, in_=xT[jb, :, 0:FB // 2])
            hb = xt_pool.tile([128, FB // 2], BF16, tag="xt", name=f"xt{jb}b")
            nc.gpsimd.dma_start(out=hb, in_=xT[jb, :, FB // 2:FB])
            parts = ((ha, 0), (hb, KT // 2))
            parts8 = ((x8_t, 0),)

        def xsl(kt):
            for t, base in reversed(parts):
                if kt >= base:
                    return t[:, (kt - base) * JBLK:(kt - base + 1) * JBLK]

        def x8sl(kt):  # [128, 2, JBLK] kt-pair slice
            for t, base in reversed(parts8):
                if kt >= base:
                    return t[:, kt - base:kt - base + 2, :]

        tok = slice(jb * JBLK, (jb + 1) * JBLK)

        ps_k = pp_pool.tile([128, JBLK], F32, tag="pp")
        for kp in range(KT // 2):
            nc.tensor.matmul(ps_k, wk8_sb[:, 2 * kp:2 * kp + 2, :],
                             x8sl(2 * kp),
                             start=(kp == 0), stop=(kp == KT // 2 - 1),
                             perf_mode=mybir.MatmulPerfMode.DoubleRow)
        nc.scalar.activation(out=kT_sb[:, tok], in_=ps_k,
                             func=mybir.ActivationFunctionType.Identity,
                             bias=b_sb[:, 1:2], scale=1.0 / W8SCALE)

        ps_v = pp_pool.tile([128, JBLK], F32, tag="pp")
        for kt in range(KT):
            nc.tensor.matmul(ps_v, w_ap(2, kt), xsl(kt),
                             start=(kt == 0), stop=(kt == KT - 1))
        vT_t = vt_pool.tile([128, JBLK], BF16, tag="vt")
        nc.vector.tensor_scalar_add(vT_t, ps_v, b_sb[:, 2:3])
        for c in range(4):
            ps_tp = pp_pool.tile([128, 512], BF16, tag="pp")
            dst = ps_tp[:, 0:128]
            nc.tensor.transpose(dst, vT_t[:, c * 128:(c + 1) * 128], ident_bf)
            jt = jb * 4 + c
            nc.vector.tensor_copy(v_sb[:, jt * 128:(jt + 1) * 128], dst)

        if jb < 2:  # Q projection for the core's own tokens (rolled blocks 0/1)
            ps_q = pp_pool.tile([128, JBLK], F32, tag="pp")
            for kt in range(KT):
                nc.tensor.matmul(ps_q, w_ap(0, kt), xsl(kt),
                                 start=(kt == 0), stop=(kt == KT - 1))
            nc.scalar.activation(out=qT_sb[:, jb * JBLK:(jb + 1) * JBLK], in_=ps_q,
                                 func=mybir.ActivationFunctionType.Identity,
                                 bias=b_sb[:, 0:1], scale=1.0)

    # The V-matmuls run one j-tile behind the S-matmuls (software
    # pipeline): the in-order PE then never stalls on exp(jt) -- V(jt-1)
    # executes while ACT computes exp(jt).
    pend = []

    def emit_v(jt, e):
        kj = slice(jt * 128, (jt + 1) * 128)
        for qb in range(NQB):
            qs = slice(qb * QBLK, (qb + 1) * QBLK)
            nc.tensor.matmul(po_t[:, qs], v_sb[:, kj], e[:, qs],
                             start=(jt == 0), stop=(jt == NJT - 1))

    def attention_block(jb):
        for c in range(4):
            jt = jb * 4 + c
            kj = slice(jt * 128, (jt + 1) * 128)
            ps_s = ps_pool.tile([128, NB], F32, tag="ps")
            for qb in range(NQB):
                qs = slice(qb * QBLK, (qb + 1) * QBLK)
                nc.tensor.matmul(ps_s[:, qs], kT_sb[:, kj], qT_sb[:, qs],
                                 start=True, stop=True)
            e = exp_pool.tile([128, NB], BF16, tag="exp")
            nc.scalar.activation(out=e, in_=ps_s,
                                 func=mybir.ActivationFunctionType.Exp,
                                 scale=SCALE)
            if jt == 0:
                nc.vector.tensor_copy(acc_all, e)
            else:
                nc.vector.tensor_add(acc_all, acc_all, e)
            if pend:
                emit_v(*pend.pop())
            pend.append((jt, e))

    # --- PE warm-up -------------------------------------------------------
    # ~4us of dummy matmuls during the initial DMA wait flips the PE HAM
    # clock gate to 8/8 before the real work arrives (PE is idle anyway).
    warm = singles.tile([128, 512], BF16, tag="warm")
    nc.vector.memset(warm, 0.0)
    ps_w = ps_pool.tile([128, NB], F32, tag="ps")
    for _ in range(20):
        nc.tensor.matmul(ps_w[:, 0:512], warm[:, 0:128], warm,
                         start=True, stop=True)

    # --- main stream ------------------------------------------------------
    stream_block(0)
    stream_block(1)
    attention_block(0)
    for jb in range(2, NJB):
        stream_block(jb)
        attention_block(jb - 1)
    attention_block(NJB - 1)
    emit_v(*pend.pop())  # flush the pipelined last V-matmul

    # --- epilogue ---------------------------------------------------------
    # denominator: sum acc over its 128 partitions via ones-matmuls, one
    # [128,1] chunk per 128 queries (lands per-partition).  acc is already
    # fp16 (accumulated at DVE 2x rate), which the PE matmuls eat directly.
    NG = NB // 128
    ps_d = ps_pool.tile([128, NB], F32, tag="ps")
    for g in range(NG):
        nc.tensor.matmul(ps_d[:, g:g + 1],
                         acc_all[:, g * 128:(g + 1) * 128], ones128,
                         start=True, stop=True)
    nc.vector.reciprocal(rden_sb, ps_d[:, 0:NG])
    dma_engs = [nc.sync, nc.scalar]
    for g in range(NB // 128):
        # O^T -> SBUF (bf16, per 128-query chunk), transpose (1 cyc/row),
        # scale by 1/den on DVE, store.  Chunked copies + per-chunk PSUM
        # tiles let the transpose->scale->store chains pipeline; the out
        # DMAs round-robin over four queues so they don't serialize.
        oT_t = oT_pool.tile([128, 128], BF16, tag="oT")
        nc.vector.tensor_copy(oT_t, po_t[:, g * 128:(g + 1) * 128])
        ps_to = pp_pool.tile([128, 512], BF16, tag="pp")
        dst = ps_to[:, 0:128]
        nc.tensor.transpose(dst, oT_t, ident_bf)
        ob = o_pool.tile([128, DH], F32, tag="o")
        nc.vector.tensor_scalar_mul(ob, dst, rden_sb[:, g:g + 1])
        dma_engs[g % 2].dma_start(out=out[g * 128:(g + 1) * 128, :], in_=ob)


def build_nc():
    if "nc" in _CACHE:
        return _CACHE["nc"]
    from contextlib import ExitStack

    nc = bacc.Bacc("TRN2", target_bir_lowering=False, debug=False,
                   num_devices=N_CORES)
    xT = nc.dram_tensor("xT", [NJB, 128, FB], BF16, kind="ExternalInput").ap()
    xT8 = nc.dram_tensor("xT8", [NJB, 128, FB], F8, kind="ExternalInput").ap()
    wk8 = nc.dram_tensor("wk8", [128, KT, 128], F8, kind="ExternalInput").ap()
    w_all = nc.dram_tensor("w_all", [128, 2 * D], BF16, kind="ExternalInput").ap()
    b_all = nc.dram_tensor("b_all", [128, 3], F32, kind="ExternalInput").ap()
    out = nc.dram_tensor("out", [NB, DH], F32, kind="ExternalOutput").ap()

    with tile.TileContext(nc) as tc:
        with ExitStack() as ctx:
            _emit(ctx, tc, nc, xT, xT8, wk8, w_all, b_all, out)
    nc.compile()
    _CACHE["nc"] = nc
    return nc


def make_in_maps(inputs):
    x = np.asarray(inputs["x"], dtype=np.float32)
    # blocked x.T: blk[jb, p, kt*JBLK + n] = x.T[kt*128 + p, jb*JBLK + n]
    #            = x[jb*JBLK + n, kt*128 + p]
    import ml_dtypes
    xb = x.reshape(NJB, JBLK, KT, 128)                    # [jb, n, kt, p]
    blkf = np.ascontiguousarray(
        xb.transpose(0, 3, 2, 1)).reshape(NJB, 128, FB)   # [jb, p, kt*n]
    blk = blkf.astype(ml_dtypes.bfloat16)
    blk8 = blk.astype(ml_dtypes.float8_e4m3)

    w_cols = []
    for wn in ("Wv", "Wq"):
        w = np.asarray(inputs[wn], np.float32)            # [D, DH]
        wr = w.reshape(KT, 128, DH).transpose(1, 0, 2).reshape(128, D)
        w_cols.append(wr)
    w_all = np.concatenate(w_cols, axis=1).astype(ml_dtypes.bfloat16)
    wk = np.asarray(inputs["Wk"], np.float32)
    wk8 = np.ascontiguousarray(
        (wk.reshape(KT, 128, DH).transpose(1, 0, 2) * W8SCALE)
        .astype(ml_dtypes.bfloat16).astype(ml_dtypes.float8_e4m3))
    b_all = np.ascontiguousarray(np.stack(
        [np.asarray(inputs[bn], np.float32) for bn in ("bq", "bk", "bv")],
        axis=1))                                          # [128, 3]

    in_maps = []
    for c in range(N_CORES):
        m = {
            "xT": np.ascontiguousarray(np.roll(blk, -2 * c, axis=0)),
            "xT8": np.ascontiguousarray(np.roll(blk8, -2 * c, axis=0)),
            "wk8": wk8,
            "w_all": w_all,
            "b_all": b_all,
        }
        in_maps.append(m)
    return in_maps


def kernel(**inputs) -> np.ndarray:
    global LAST_RESULTS
    nc = build_nc()
    in_maps = make_in_maps(inputs)
    try:
        res = run_bass_kernel_spmd(nc, in_maps, core_ids=list(range(N_CORES)))
    except Exception:
        # The device occasionally reports a transient unrecoverable exec
        # state (observed NRT_EXEC_UNIT_UNRECOVERABLE); a single retry has
        # been seen to clear it and costs nothing on the happy path.
        res = run_bass_kernel_spmd(nc, in_maps, core_ids=list(range(N_CORES)))
    LAST_RESULTS = res
    return np.concatenate([res.results[c]["out"] for c in range(N_CORES)],
                          axis=0)

